# revision 1
# baseline (speedup 1.0000x reference)
"""Trainium2 Bass kernel for nn_Encoder_21371757265491.

Math (reference.py):
  stage 1: per-(b,t) one-step LSTM from zero state:
      gates = X @ W_ih1.T + (b_ih1+b_hh1); c = sig(i)*tanh(g); h = sig(o)*tanh(c)
  stage 2: A[b,t,s] = concat(h,c) @ W_we.T + b_we ; U[b,d,s] = sum_t X[b,t,d] W_ue[s,t] + b_ue
  stage 3: score[b,t,d] = sum_s v_s tanh(A[b,t,s]+U[b,d,s]) (+bv, cancels in softmax)
           Xt[b,t,d] = softmax_d(score) * X[b,t,d]
  stage 4: LSTM scanning over b (seq-first bug), batch dim = t.

Sharding: stages 1-3 data-parallel over B (32 b/core); AllToAll; stage 4
sharded over the t axis (32 lanes/core).
"""

import numpy as np

B, T, D, H = 256, 256, 128, 128
NC = 8
BPC = B // NC  # b per core, stages 1-3
TPC = T // NC  # t lanes per core, stage 4
TG = 16        # t-group for stage-3 arg/z tiles (free dim = TG*D = 2048)
SG = 8         # t per PSUM score strip (strip free = SG*D = 1024)
S4G = 32       # stage-4 b-group for the x-part precompute

_CACHE = {}


def _build(nb=BPC, nt=T, run_s4=True, z_bf16=True, dbg=True, reps=1,
           dot_stride=1, skip_add=False, skip_tanh=False):
    import concourse.bass as bass
    import concourse.bacc as bacc
    import concourse.mybir as mybir
    from concourse import tile

    f32 = mybir.dt.float32
    AF = mybir.ActivationFunctionType
    ALU = mybir.AluOpType
    assert nt % TG == 0 and nt % SG == 0

    nc = bacc.Bacc("TRN2", target_bir_lowering=False, debug=False, num_devices=NC)

    # ---------------- DRAM I/O ----------------
    X_d = nc.dram_tensor("x", [nb, nt, D], f32, kind="ExternalInput").ap()
    # stage-1 weights: W_ih1.T columns for gates (i, g, o), biases pre-halved
    w1t_d = nc.dram_tensor("w1t", [D, 3 * H], f32, kind="ExternalInput").ap()
    b1h_d = nc.dram_tensor("b1h", [H, 3], f32, kind="ExternalInput").ap()
    # stage-2: W_we.T * 0.5 as [2][128 (j), 256 (s)], W_ue.T as [2][128 (t), 256 (s)]
    wwet_d = nc.dram_tensor("wwet", [2, H, 2 * H], f32, kind="ExternalInput").ap()
    bwe_d = nc.dram_tensor("bwe", [H, 2], f32, kind="ExternalInput").ap()
    wuet_d = nc.dram_tensor("wuet", [2, H, 2 * H], f32, kind="ExternalInput").ap()
    bue_d = nc.dram_tensor("bue", [H, 2], f32, kind="ExternalInput").ap()
    v_d = nc.dram_tensor("v", [H, 2], f32, kind="ExternalInput").ap()
    ident_d = nc.dram_tensor("ident", [128, 128], f32, kind="ExternalInput").ap()
    # stage-4 (gate rows permuted to i,f,o,g): W.T [128, 512], bias row [1, 512]
    wih2t_d = nc.dram_tensor("wih2t", [D, 4 * H], f32, kind="ExternalInput").ap()
    whh2t_d = nc.dram_tensor("whh2t", [H, 4 * H], f32, kind="ExternalInput").ap()
    b2_d = nc.dram_tensor("b2", [1, 4 * H], f32, kind="ExternalInput").ap()
    ones_d = nc.dram_tensor("ones", [1, 512], f32, kind="ExternalInput").ap()
    vb_d = nc.dram_tensor("vb", [H, 2], mybir.dt.bfloat16, kind="ExternalInput").ap()
    onescol_d = nc.dram_tensor("onescol", [H, 1], f32, kind="ExternalInput").ap()

    cc_in = nc.dram_tensor("cc_in", [NC, nb, D, TPC], f32).ap()
    cc_out = nc.dram_tensor("cc_out", [NC, nb, D, TPC], f32).ap()
    y_d = nc.dram_tensor("y", [B, H, TPC], f32, kind="ExternalOutput").ap()
    xt_dbg = (
        nc.dram_tensor("xt_dbg", [nb, D, nt], f32, kind="ExternalOutput").ap()
        if dbg else None
    )

    NTG = nt // TG      # t-groups per b
    NSG = nt // SG      # strips per b
    NTH = (nt + 127) // 128  # t-halves (128-sized) per b

    with tile.TileContext(nc) as tc:
        # ---------------- constant pools ----------------
        with tc.tile_pool(name="const", bufs=1) as cpool:
            w1t_s = cpool.tile([D, 3 * H], f32, tag="w1t", name="w1t")
            nc.sync.dma_start(out=w1t_s[:], in_=w1t_d)
            b1h_s = cpool.tile([H, 3], f32, tag="b1h", name="b1h")
            nc.sync.dma_start(out=b1h_s[:], in_=b1h_d)
            wwet_s = [cpool.tile([H, 2 * H], f32, tag=f"wwet{j}", name=f"wwet{j}") for j in range(2)]
            for j in range(2):
                nc.sync.dma_start(out=wwet_s[j][:], in_=wwet_d[j])
            bwe_s = cpool.tile([H, 2], f32, tag="bwe", name="bwe")
            nc.sync.dma_start(out=bwe_s[:], in_=bwe_d)
            wuet_s = [cpool.tile([H, 2 * H], f32, tag=f"wuet{j}", name=f"wuet{j}") for j in range(2)]
            for j in range(2):
                nc.sync.dma_start(out=wuet_s[j][:], in_=wuet_d[j])
            bue_s = cpool.tile([H, 2], f32, tag="bue", name="bue")
            nc.sync.dma_start(out=bue_s[:], in_=bue_d)
            v_s = cpool.tile([H, 2], f32, tag="v", name="v")
            nc.sync.dma_start(out=v_s[:], in_=v_d)
            ident_s = cpool.tile([128, 128], f32, tag="ident", name="ident")
            nc.sync.dma_start(out=ident_s[:], in_=ident_d)
            wih2t_s = cpool.tile([D, 4 * H], f32, tag="wih2t", name="wih2t")
            nc.sync.dma_start(out=wih2t_s[:], in_=wih2t_d)
            whh2t_s = cpool.tile([H, 4 * H], f32, tag="whh2t", name="whh2t")
            nc.sync.dma_start(out=whh2t_s[:], in_=whh2t_d)
            b2_s = cpool.tile([1, 4 * H], f32, tag="b2", name="b2")
            nc.sync.dma_start(out=b2_s[:], in_=b2_d)
            ones_s = cpool.tile([1, 512], f32, tag="ones", name="ones")
            nc.sync.dma_start(out=ones_s[:], in_=ones_d)
            vb_s = cpool.tile([H, 2], mybir.dt.bfloat16, tag="vb", name="vb")
            nc.sync.dma_start(out=vb_s[:], in_=vb_d)
            onescol_s = cpool.tile([H, 1], f32, tag="onescol", name="onescol")
            nc.sync.dma_start(out=onescol_s[:], in_=onescol_d)

            # ---------------- stages 1-3 ----------------
            with (
                tc.tile_pool(name="sb13", bufs=2) as sb,
                tc.tile_pool(name="zpool", bufs=(2 if z_bf16 else 1)) as zp,
                tc.tile_pool(name="sb13b", bufs=3) as sb3,
                tc.tile_pool(name="ps_mm1", bufs=1, space="PSUM") as pmm1,
                tc.tile_pool(name="ps_g1", bufs=1, space="PSUM") as pg1,
                tc.tile_pool(name="ps_mm2", bufs=2, space="PSUM") as pmm2,
            ):
                for bb in range(reps * nb):
                    b = bb % nb
                    # -- load X_b natural [t, d] as t-half tiles
                    xn = []
                    for th in range(NTH):
                        t0 = th * 128
                        xt_ = sb3.tile([128, D], f32, tag="xnat", name="xnat")
                        nc.sync.dma_start(out=xt_[:], in_=X_d[b, t0 : t0 + 128, :])
                        xn.append(xt_)
                    # -- X^T [d, t] via PE transpose
                    xT = sb.tile([D, nt], f32, tag="xT", name="xT")
                    for th in range(NTH):
                        pt = pmm1.tile([128, 128], f32, tag="pt_xu", name="pt_xpose")
                        nc.tensor.transpose(pt[:], xn[th][:], ident_s[:])
                        nc.scalar.copy(xT[:, th * 128 : (th + 1) * 128], pt[:])
                    # -- stage 1 gates^T: [H, t] per gate (i, g, o)
                    g1 = pg1.tile([H, 3 * nt], f32, tag="g1", name="g1")
                    for gi in range(3):
                        nc.tensor.matmul(
                            g1[:, gi * nt : (gi + 1) * nt],
                            w1t_s[:, gi * H : (gi + 1) * H],
                            xT[:],
                            start=True,
                            stop=True,
                        )
                    # tanh-trick: sig(x) = 0.5 + 0.5*tanh(x/2)
                    # t_i = tanh(0.5*g1_i + b1h_i)  (b1h = 0.5*(b_ih1+b_hh1))
                    ti = sb.tile([H, nt], f32, tag="ti", name="ti")
                    nc.scalar.activation(
                        ti[:], g1[:, 0:nt], AF.Tanh, bias=b1h_s[:, 0:1], scale=0.5
                    )
                    tg = sb.tile([H, nt], f32, tag="tg", name="tg")
                    nc.scalar.activation(
                        tg[:], g1[:, nt : 2 * nt], AF.Tanh, bias=b1h_s[:, 1:2], scale=0.5
                    )
                    to = sb.tile([H, nt], f32, tag="to", name="to")
                    nc.scalar.activation(
                        to[:], g1[:, 2 * nt : 3 * nt], AF.Tanh, bias=b1h_s[:, 2:3], scale=0.5
                    )
                    # c' = 2c = (1+t_i)*t_g = t_g + t_i*t_g
                    cp = sb.tile([H, nt], f32, tag="cp", name="cp")
                    nc.vector.tensor_mul(cp[:], ti[:], tg[:])
                    nc.vector.tensor_add(cp[:], cp[:], tg[:])
                    # t_c = tanh(c) = tanh(0.5 * c')
                    tc_ = sb.tile([H, nt], f32, tag="tc", name="tc")
                    nc.scalar.activation(tc_[:], cp[:], AF.Tanh, scale=0.5)
                    # h' = 2h = (1+t_o)*t_c
                    hp = sb.tile([H, nt], f32, tag="hp", name="hp")
                    nc.vector.tensor_mul(hp[:], to[:], tc_[:])
                    nc.vector.tensor_add(hp[:], hp[:], tc_[:])

                    # -- stage 2: A^T [s, t]  (wwet_s already scaled by 0.5)
                    aT = [sb.tile([H, nt], f32, tag=f"aT{sc}", name=f"aT{sc}") for sc in range(2)]
                    for sc in range(2):
                        pa = pmm1.tile([128, nt], f32, tag="pt_a", name="pt_a")
                        nc.tensor.matmul(
                            pa[:], wwet_s[0][:, sc * 128 : (sc + 1) * 128], hp[:],
                            start=True, stop=False,
                        )
                        nc.tensor.matmul(
                            pa[:], wwet_s[1][:, sc * 128 : (sc + 1) * 128], cp[:],
                            start=False, stop=True,
                        )
                        nc.scalar.add(aT[sc][:], pa[:], bwe_s[:, sc : sc + 1])
                    # -- stage 2: U^T [s, d]
                    uT = [sb.tile([H, D], f32, tag=f"uT{sc}", name=f"uT{sc}") for sc in range(2)]
                    for sc in range(2):
                        pu = pmm1.tile([128, D], f32, tag="pt_xu", name="pt_u")
                        for th in range(NTH):
                            nc.tensor.matmul(
                                pu[:],
                                wuet_s[th][:, sc * 128 : (sc + 1) * 128],
                                xn[th][:],
                                start=(th == 0),
                                stop=(th == NTH - 1),
                            )
                        nc.scalar.add(uT[sc][:], pu[:], bue_s[:, sc : sc + 1])

                    # -- stage 3
                    bf16 = mybir.dt.bfloat16
                    zdt = bf16 if z_bf16 else f32
                    vdot_s = vb_s if z_bf16 else v_s
                    for th in range(NTH):
                        t0 = th * 128
                        # z tiles (bf16) for this t-half: [s-chunk][128, 128*D]
                        zt = []
                        for scn in range(2):
                            z = zp.tile([128, 128 * D], zdt, tag=f"z{scn}", name=f"z{scn}")
                            zt.append(z)
                        for gg in range(128 // TG):
                            tg0 = t0 + gg * TG
                            for scn in range(2):
                                arg = sb.tile([128, TG * D], f32, tag=f"arg{scn}", name=f"arg{scn}")
                                a_sl = (
                                    aT[scn][:, tg0 : tg0 + TG]
                                    .unsqueeze(2)
                                    .broadcast_to([128, TG, D])
                                )
                                u_sl = (
                                    uT[scn][:]
                                    .unsqueeze(1)
                                    .broadcast_to([128, TG, D])
                                )
                                argv = arg[:].rearrange("p (t d) -> p t d", d=D)
                                if not skip_add:
                                    nc.vector.tensor_add(argv, a_sl, u_sl)
                                else:
                                    nc.vector.memset(arg[:, 0:1], 0.0)
                                if not skip_tanh:
                                    nc.scalar.activation(
                                        zt[scn][:, gg * TG * D : (gg + 1) * TG * D],
                                        arg[:],
                                        AF.Tanh,
                                    )
                                else:
                                    nc.vector.memset(
                                        zt[scn][:, gg * TG * D : gg * TG * D + 2], 0.0
                                    )
                        # dot: z_t [s,d] as stationary weight, v as 1-col moving
                        scT = pmm2.tile([128, 128], f32, tag="sc_rb", name="scT")
                        for tl in range(0, 128, dot_stride):
                            for scn in range(2):
                                nc.tensor.matmul(
                                    scT[:, tl : tl + 1],
                                    zt[scn][:, tl * D : (tl + 1) * D],
                                    vdot_s[:, scn : scn + 1],
                                    start=(scn == 0),
                                    stop=(scn == 1),
                                )
                        # E = exp(scores^T) [d, t]
                        esb = sb.tile([128, 128], f32, tag="esb", name="esb")
                        nc.scalar.activation(esb[:], scT[:], AF.Exp)
                        # column sums over d via ones-dot: sums [t, 1]
                        sums = pmm2.tile([128, 1], f32, tag="sr", name="sums")
                        nc.tensor.matmul(
                            sums[:], esb[:], onescol_s[:], start=True, stop=True
                        )
                        rT = sb.tile([128, 1], f32, tag="rT", name="rT")
                        nc.vector.reciprocal(rT[:], sums[:])
                        # transpose r to a row, broadcast to [d, t] via rank-1 matmul
                        r_ps = pmm2.tile([1, 128], f32, tag="sr", name="r_ps")
                        nc.tensor.transpose(r_ps[:], rT[:], ident_s[:])
                        r_row = sb.tile([1, 128], f32, tag="r_row", name="r_row")
                        nc.scalar.copy(r_row[:], r_ps[:])
                        rbc = pmm2.tile([128, 128], f32, tag="sc_rb", name="rbc")
                        nc.tensor.matmul(
                            rbc[:], ones_s[0:1, 0:128], r_row[:], start=True, stop=True
                        )
                        # Xt^T[d, t] = E * rbc * X^T
                        w1_ = sb.tile([128, 128], f32, tag="w1_", name="w1_")
                        nc.vector.tensor_mul(w1_[:], esb[:], rbc[:])
                        xtT = sb.tile([128, 128], f32, tag="xtT", name="xtT")
                        nc.vector.tensor_mul(
                            xtT[:], w1_[:], xT[:, t0 : t0 + 128]
                        )
                        if dbg:
                            nc.sync.dma_start(
                                out=xt_dbg[b, :, t0 : t0 + 128], in_=xtT[:]
                            )
                        # ship transposed lane-blocks to cc_in
                        for q in range(128 // TPC):
                            j = t0 // TPC + q
                            nc.sync.dma_start(
                                out=cc_in[j, b, :, :],
                                in_=xtT[:, q * TPC : (q + 1) * TPC],
                            )

            for _rep in range(reps if run_s4 else 0):
                # ---------------- AllToAll ----------------
                nc.gpsimd.collective_compute(
                    "AllToAll",
                    ALU.bypass,
                    replica_groups=[list(range(NC))],
                    ins=[cc_in],
                    outs=[cc_out],
                )

                # ---------------- stage 4 ----------------
                with (
                    tc.tile_pool(name="sb4", bufs=2) as sb4,
                    tc.tile_pool(name="sb4c", bufs=1) as sb4c,
                    tc.tile_pool(name="ps4", bufs=1, space="PSUM") as ps4,
                ):
                    xTt = sb4c.tile([D, B * TPC], f32, tag="xTt", name="xTt")
                    for i in range(NC):
                        nc.sync.dma_start(
                            out=xTt[:, i * nb * TPC : (i + 1) * nb * TPC].rearrange(
                                "d (b l) -> d b l", l=TPC
                            ),
                            in_=cc_out[i].rearrange("b d l -> d b l"),
                        )

                    ctiles = [sb4c.tile([H, TPC], f32, tag=f"c{i}", name=f"c{i}") for i in range(2)]
                    htiles = [sb4c.tile([H, TPC], f32, tag=f"h{i}", name=f"h{i}") for i in range(2)]
                    nc.vector.memset(ctiles[0][:], 0.0)
                    nc.vector.memset(htiles[0][:], 0.0)

                    NBG = B // S4G
                    for bg in range(NBG):
                        # x-part + bias for this b-group, PSUM layout
                        # [128, (chunk, b_local, lane)] chunk-major
                        p1 = ps4.tile([128, 4 * S4G * TPC], f32, tag="p1", name="p1")
                        for c in range(4):
                            for sub in range(S4G * TPC // 512):
                                o = c * S4G * TPC + sub * 512
                                nc.tensor.matmul(
                                    p1[:, o : o + 512],
                                    wih2t_s[:, c * 128 : (c + 1) * 128],
                                    xTt[:, bg * S4G * TPC + sub * 512 : bg * S4G * TPC + (sub + 1) * 512],
                                    start=True, stop=False,
                                    skip_group_check=True,
                                )
                                nc.tensor.matmul(
                                    p1[:, o : o + 512],
                                    b2_s[0:1, c * 128 : (c + 1) * 128],
                                    ones_s[0:1, :],
                                    start=False, stop=False,
                                    skip_group_check=True,
                                )
                        for bl in range(S4G):
                            b = bg * S4G + bl
                            hprev = htiles[b % 2]
                            cprev = ctiles[b % 2]
                            hcur = htiles[1 - b % 2]
                            ccur = ctiles[1 - b % 2]
                            # hh-part accumulated into p1 slice of this b
                            for c in range(4):
                                o = c * S4G * TPC + bl * TPC
                                nc.tensor.matmul(
                                    p1[:, o : o + TPC],
                                    whh2t_s[:, c * 128 : (c + 1) * 128],
                                    hprev[:],
                                    start=False, stop=(c == 3),
                                    skip_group_check=True,
                                )
                            g2 = p1[:].rearrange("p (c b l) -> p c b l", c=4, b=S4G)
                            # gates (rows permuted i,f,o,g): sigmoid on c=0..2
                            sig = sb4.tile([H, 3 * TPC], f32, tag="sig", name="sig")
                            nc.scalar.activation(
                                sig[:].rearrange("p (c l) -> p c l", c=3),
                                g2[:, 0:3, bl, :],
                                AF.Sigmoid,
                            )
                            tg4 = sb4.tile([H, TPC], f32, tag="tg4", name="tg4")
                            nc.scalar.activation(tg4[:], g2[:, 3, bl, :], AF.Tanh)
                            # c = sig_f*c_prev + sig_i*tg
                            t1 = sb4.tile([H, TPC], f32, tag="t1", name="t1")
                            nc.vector.tensor_mul(t1[:], sig[:, 0:TPC], tg4[:])
                            nc.vector.tensor_mul(
                                ccur[:], sig[:, TPC : 2 * TPC], cprev[:]
                            )
                            nc.vector.tensor_add(ccur[:], ccur[:], t1[:])
                            tc4 = sb4.tile([H, TPC], f32, tag="tc4", name="tc4")
                            nc.scalar.activation(tc4[:], ccur[:], AF.Tanh)
                            nc.vector.tensor_mul(
                                hcur[:], sig[:, 2 * TPC : 3 * TPC], tc4[:]
                            )
                            nc.sync.dma_start(out=y_d[b, :, :], in_=hcur[:])

    nc.compile()
    return nc


def _get_nc(key, **kw):
    if key not in _CACHE:
        _CACHE[key] = _build(**kw)
    return _CACHE[key]


KERNEL_VARIANT = {"z_bf16": True}


def _prep_weights(W_ih1, b_ih1, W_hh1, b_hh1, W_we, b_we, W_ue, b_ue, W_ve, b_ve,
                  W_ih2, b_ih2, W_hh2, b_hh2):
    f = np.float32
    b1 = (b_ih1 + b_hh1).astype(f)
    # gate order torch: i, f, g, o ; we need i, g, o
    w1t = np.concatenate(
        [W_ih1[0:H].T, W_ih1[2 * H : 3 * H].T, W_ih1[3 * H : 4 * H].T], axis=1
    ).astype(f)  # [D, 3H]
    b1h = 0.5 * np.stack(
        [b1[0:H], b1[2 * H : 3 * H], b1[3 * H : 4 * H]], axis=1
    ).astype(f)  # [H, 3]
    wwet = (0.5 * W_we.T).reshape(2, H, 2 * H).astype(f)  # [j-half][j128][s256]
    bwe = b_we.reshape(2, H).T.copy().astype(f)  # [H, 2] column per s-chunk
    wuet = W_ue.T.reshape(2, H, 2 * H).astype(f)  # [t-half][t128][s256]
    bue = b_ue.reshape(2, H).T.copy().astype(f)
    v = W_ve[0].reshape(2, H).T.copy().astype(f)  # [H, 2]
    ident = np.eye(128, dtype=f)
    # stage 4: permute gates to (i, f, o, g)
    perm = np.concatenate(
        [np.arange(0, H), np.arange(H, 2 * H), np.arange(3 * H, 4 * H),
         np.arange(2 * H, 3 * H)]
    )
    wih2t = W_ih2[perm].T.copy().astype(f)  # [D, 4H]
    whh2t = W_hh2[perm].T.copy().astype(f)  # [H, 4H]
    b2 = (b_ih2 + b_hh2)[perm].reshape(1, 4 * H).astype(f)
    ones = np.ones((1, 512), dtype=f)
    vb = v.astype(np.dtype('bfloat16')) if hasattr(np, 'bfloat16') else None
    import ml_dtypes
    vb = v.astype(ml_dtypes.bfloat16)
    onescol = np.ones((H, 1), dtype=f)
    return dict(
        w1t=w1t, b1h=b1h, wwet=wwet, bwe=bwe, wuet=wuet, bue=bue, v=v,
        ident=ident, wih2t=wih2t, whh2t=whh2t, b2=b2, ones=ones,
        vb=vb, onescol=onescol,
    )


def kernel(X, W_ih1, b_ih1, W_hh1, b_hh1, W_we, b_we, W_ue, b_ue, W_ve, b_ve,
           W_ih2, b_ih2, W_hh2, b_hh2, _trace=False):
    from concourse.bass_utils import run_bass_kernel_spmd

    X = np.asarray(X, dtype=np.float32)
    wd = _prep_weights(
        np.asarray(W_ih1), np.asarray(b_ih1), np.asarray(W_hh1), np.asarray(b_hh1),
        np.asarray(W_we), np.asarray(b_we), np.asarray(W_ue), np.asarray(b_ue),
        np.asarray(W_ve), np.asarray(b_ve), np.asarray(W_ih2), np.asarray(b_ih2),
        np.asarray(W_hh2), np.asarray(b_hh2),
    )
    nc = _get_nc(("full", tuple(sorted(KERNEL_VARIANT.items()))), **KERNEL_VARIANT)
    in_maps = [
        {"x": np.ascontiguousarray(X[k * BPC : (k + 1) * BPC]), **wd}
        for k in range(NC)
    ]
    res = run_bass_kernel_spmd(nc, in_maps, core_ids=list(range(NC)), trace=_trace)
    out = np.empty((B, T, H), dtype=np.float32)
    for k in range(NC):
        out[:, k * TPC : (k + 1) * TPC, :] = res.results[k]["y"].transpose(0, 2, 1)
    if _trace:
        kernel.last_result = res
    return out



# revision 5
# speedup vs baseline: 3.3529x; 3.3529x over previous
"""Trainium2 Bass kernel for nn_Encoder_21371757265491.

Math (reference.py):
  stage 1: per-(b,t) one-step LSTM from zero state:
      gates = X @ W_ih1.T + (b_ih1+b_hh1); c = sig(i)*tanh(g); h = sig(o)*tanh(c)
  stage 2: A[b,t,s] = concat(h,c) @ W_we.T + b_we ; U[b,d,s] = sum_t X[b,t,d] W_ue[s,t] + b_ue
  stage 3: score[b,t,d] = sum_s v_s tanh(A[b,t,s]+U[b,d,s]) (+bv, cancels in softmax)
           Xt[b,t,d] = softmax_d(score) * X[b,t,d]
  stage 4: LSTM scanning over b (seq-first bug), batch dim = t.

Key trick (stage 3): tanh(x) ~= sum_k s_k sin(k*w0*x), and
  sin(w(a+u)) = sin(wa)cos(wu) + cos(wa)sin(wu)
is separable, so the t*d*s elementwise tanh becomes 2K bf16 matmuls on the
PE.  Harmonics are built from in-range Act-engine Sin seeds (|arg| <= pi)
via Chebyshev-style recurrences split across Act (squares/affines) and
DVE (products).

Sharding: stages 1-3 data-parallel over B (32 b/core); bf16 AllToAll;
stage 4 sharded over the t axis (32 lanes/core), two independent
lane-chains to pipeline the sequential scan.
"""

import math

import numpy as np

B, T, D, H = 256, 256, 128, 128
NC = 8
BPC = B // NC  # b per core, stages 1-3
TPC = T // NC  # t lanes per core, stage 4
KH = 4         # sine harmonics
LFIT = 5.5     # sine half-period for the tanh fit
S4G = 16       # stage-4 b-group for the x-part precompute
NCH = 2        # stage-4 independent t-lane chains
LCH = TPC // NCH

_CACHE = {}


def _fit_coeffs():
    om = np.pi / LFIT * np.arange(1, KH + 1)
    xg = np.linspace(-5.2, 5.2, 20001)
    wgt = np.exp(-(xg ** 2) / (2 * 0.82 ** 2)) + 1e-3
    phi = np.sin(np.outer(xg, om))
    coef, *_ = np.linalg.lstsq(
        phi * np.sqrt(wgt)[:, None], np.tanh(xg) * np.sqrt(wgt), rcond=None
    )
    return om, coef


def _build(nb=BPC):
    import concourse.bass as bass
    import concourse.bacc as bacc
    import concourse.mybir as mybir
    from concourse import tile

    f32 = mybir.dt.float32
    bf16 = mybir.dt.bfloat16
    AF = mybir.ActivationFunctionType
    ALU = mybir.AluOpType
    W0 = math.pi / LFIT
    nbB = nb * NC  # global batch (stage-4 scan length)
    assert nbB % S4G == 0

    nc = bacc.Bacc("TRN2", target_bir_lowering=False, debug=False, num_devices=NC)

    # ---------------- DRAM I/O ----------------
    xnat_d = nc.dram_tensor("xnat", [nb, T, D], bf16, kind="ExternalInput").ap()
    xtr_d = nc.dram_tensor("xtr", [nb, D, T], bf16, kind="ExternalInput").ap()
    w1t_d = nc.dram_tensor("w1t", [D, 3 * H], bf16, kind="ExternalInput").ap()
    b1r_d = nc.dram_tensor("b1r", [1, 3 * H], bf16, kind="ExternalInput").ap()
    wwet_d = nc.dram_tensor("wwet", [2, H, 2 * H], bf16, kind="ExternalInput").ap()
    bwer_d = nc.dram_tensor("bwer", [1, 2 * H], bf16, kind="ExternalInput").ap()
    wuet_d = nc.dram_tensor("wuet", [2, H, 2 * H], bf16, kind="ExternalInput").ap()
    buer_d = nc.dram_tensor("buer", [1, 2 * H], bf16, kind="ExternalInput").ap()
    vsk_d = nc.dram_tensor("vsk", [H, 2 * KH], f32, kind="ExternalInput").ap()
    affc_d = nc.dram_tensor("affc", [H, 3], f32, kind="ExternalInput").ap()
    onesr_d = nc.dram_tensor("onesr", [1, 512], bf16, kind="ExternalInput").ap()
    onesc_d = nc.dram_tensor("onesc", [H, 1], bf16, kind="ExternalInput").ap()
    wih2t_d = nc.dram_tensor("wih2t", [D, 4 * H], bf16, kind="ExternalInput").ap()
    whh2t_d = nc.dram_tensor("whh2t", [H, 4 * H], bf16, kind="ExternalInput").ap()
    b2r_d = nc.dram_tensor("b2r", [1, 4 * H], bf16, kind="ExternalInput").ap()

    cc_in = nc.dram_tensor("cc_in", [NC, nb, D, TPC], bf16).ap()
    cc_out = nc.dram_tensor("cc_out", [NC, nb, D, TPC], bf16).ap()
    y_d = nc.dram_tensor("y", [nbB, H, TPC], bf16, kind="ExternalOutput").ap()

    with tile.TileContext(nc) as tc:
        with tc.tile_pool(name="const", bufs=1) as cpool:
            def cload(name, dram, shape, dt):
                t = cpool.tile(shape, dt, tag=name, name=name)
                nc.sync.dma_start(out=t[:], in_=dram)
                return t

            w1t_s = cload("w1t", w1t_d, [D, 3 * H], bf16)
            b1r_s = cload("b1r", b1r_d, [1, 3 * H], bf16)
            wwet_s = [cload(f"wwet{j}", wwet_d[j], [H, 2 * H], bf16) for j in range(2)]
            bwer_s = cload("bwer", bwer_d, [1, 2 * H], bf16)
            wuet_s = [cload(f"wuet{j}", wuet_d[j], [H, 2 * H], bf16) for j in range(2)]
            buer_s = cload("buer", buer_d, [1, 2 * H], bf16)
            vsk_s = cload("vsk", vsk_d, [H, 2 * KH], f32)
            affc_s = cload("affc", affc_d, [H, 3], f32)
            B1 = affc_s[:, 0:1]   # +1.0
            B3 = affc_s[:, 1:2]   # +3.0
            BM3 = affc_s[:, 2:3]  # -3.0
            onesr_s = cload("onesr", onesr_d, [1, 512], bf16)
            onesc_s = cload("onesc", onesc_d, [H, 1], bf16)
            wih2t_s = cload("wih2t", wih2t_d, [D, 4 * H], bf16)
            whh2t_s = cload("whh2t", whh2t_d, [H, 4 * H], bf16)
            b2r_s = cload("b2r", b2r_d, [1, 4 * H], bf16)

            # ---------------- stages 1-3 ----------------
            with (
                tc.tile_pool(name="sb13", bufs=2) as sb,
                tc.tile_pool(name="ps_g1", bufs=1, space="PSUM") as pg1,
                tc.tile_pool(name="ps_a", bufs=1, space="PSUM") as ppa,
                tc.tile_pool(name="ps_u", bufs=1, space="PSUM") as ppu,
                tc.tile_pool(name="ps_sc", bufs=2, space="PSUM") as psc,
                tc.tile_pool(name="ps_sm", bufs=1, space="PSUM") as psm,
            ):
                for b in range(nb):
                    # ---- loads
                    xn = []
                    for th in range(2):
                        t_ = sb.tile([128, D], bf16, tag=f"xn{th}", name=f"xn{th}")
                        nc.sync.dma_start(
                            out=t_[:], in_=xnat_d[b, th * 128 : (th + 1) * 128, :]
                        )
                        xn.append(t_)
                    xT = sb.tile([D, T], bf16, tag="xT", name="xT")
                    nc.sync.dma_start(out=xT[:], in_=xtr_d[b])

                    # ---- stage 1: gates^T [H, (gate, t)], gate = (i, 2g, o)
                    g1 = pg1.tile([H, 3 * T], f32, tag="g1", name="g1")
                    for gi in range(3):
                        nc.tensor.matmul(
                            g1[:, gi * T : (gi + 1) * T],
                            w1t_s[:, gi * H : (gi + 1) * H],
                            xT[:],
                            start=True, stop=False,
                        )
                        nc.tensor.matmul(
                            g1[:, gi * T : (gi + 1) * T],
                            b1r_s[0:1, gi * H : (gi + 1) * H],
                            onesr_s[0:1, 0:T],
                            start=False, stop=True,
                        )
                    tio = sb.tile([H, 3 * T], bf16, tag="tio", name="tio")
                    nc.scalar.activation(tio[:], g1[:], AF.Tanh, scale=0.5)
                    c2t = sb.tile([H, T], bf16, tag="c2t", name="c2t")
                    nc.vector.scalar_tensor_tensor(
                        c2t[:], tio[:, 0:T], 1.0, tio[:, T : 2 * T],
                        ALU.add, ALU.mult,
                    )
                    tct = sb.tile([H, T], bf16, tag="tct", name="tct")
                    nc.scalar.activation(tct[:], c2t[:], AF.Tanh, scale=0.5)
                    h2t = sb.tile([H, T], bf16, tag="h2t", name="h2t")
                    nc.vector.scalar_tensor_tensor(
                        h2t[:], tio[:, 2 * T : 3 * T], 1.0, tct[:],
                        ALU.add, ALU.mult,
                    )

                    # ---- stage 2: A^T psum [s-chunk part, (chunk, t)]
                    pa = ppa.tile([128, 2 * T], f32, tag="pa", name="pa")
                    for sc in range(2):
                        nc.tensor.matmul(
                            pa[:, sc * T : (sc + 1) * T],
                            wwet_s[0][:, sc * 128 : (sc + 1) * 128],
                            h2t[:], start=True, stop=False,
                        )
                        nc.tensor.matmul(
                            pa[:, sc * T : (sc + 1) * T],
                            wwet_s[1][:, sc * 128 : (sc + 1) * 128],
                            c2t[:], start=False, stop=False,
                        )
                        nc.tensor.matmul(
                            pa[:, sc * T : (sc + 1) * T],
                            bwer_s[0:1, sc * H : (sc + 1) * H],
                            onesr_s[0:1, 0:T],
                            start=False, stop=True,
                        )
                    # ---- stage 2: U^T psum [s-chunk part, (chunk, d)]
                    pu = ppu.tile([128, 2 * D], f32, tag="pu", name="pu")
                    for sc in range(2):
                        for th in range(2):
                            nc.tensor.matmul(
                                pu[:, sc * D : (sc + 1) * D],
                                wuet_s[th][:, sc * 128 : (sc + 1) * 128],
                                xn[th][:],
                                start=(th == 0), stop=False,
                            )
                        nc.tensor.matmul(
                            pu[:, sc * D : (sc + 1) * D],
                            buer_s[0:1, sc * H : (sc + 1) * H],
                            onesr_s[0:1, 0:D],
                            start=False, stop=True,
                        )

                    # ---- trig ladder, A side ([128, 512] bf16 tiles)
                    def anew(tag):
                        return sb.tile([128, 2 * T], bf16, tag=tag, name=tag)

                    sA = {}; cA = {}
                    shA = anew("shA")
                    nc.scalar.activation(shA[:], pa[:], AF.Sin, scale=0.5 * W0)
                    sA[1] = anew("s1A")
                    nc.scalar.activation(sA[1][:], pa[:], AF.Sin, scale=W0)
                    qA = anew("qA")
                    nc.scalar.activation(qA[:], shA[:], AF.Square)
                    cA[1] = anew("c1A")
                    nc.scalar.activation(cA[1][:], qA[:], AF.Identity, bias=B1, scale=-2.0)
                    sA[2] = anew("s2A")
                    nc.vector.scalar_tensor_tensor(
                        sA[2][:], sA[1][:], 2.0, cA[1][:], ALU.mult, ALU.mult
                    )
                    q1A = anew("q1A")
                    nc.scalar.activation(q1A[:], sA[1][:], AF.Square)
                    cA[2] = anew("c2A")
                    nc.scalar.activation(cA[2][:], q1A[:], AF.Identity, bias=B1, scale=-2.0)
                    wA = anew("wA")
                    nc.scalar.activation(wA[:], q1A[:], AF.Identity, bias=B3, scale=-4.0)
                    sA[3] = anew("s3A")
                    nc.vector.tensor_mul(sA[3][:], wA[:], sA[1][:])
                    q2A = anew("q2A")
                    nc.scalar.activation(q2A[:], cA[1][:], AF.Square)
                    w2A = anew("w2A")
                    nc.scalar.activation(w2A[:], q2A[:], AF.Identity, bias=BM3, scale=4.0)
                    cA[3] = anew("c3A")
                    nc.vector.tensor_mul(cA[3][:], w2A[:], cA[1][:])
                    sA[4] = anew("s4A")
                    nc.vector.scalar_tensor_tensor(
                        sA[4][:], sA[2][:], 2.0, cA[2][:], ALU.mult, ALU.mult
                    )
                    q3A = anew("q3A")
                    nc.vector.tensor_mul(q3A[:], sA[2][:], sA[2][:])
                    cA[4] = anew("c4A")
                    nc.vector.tensor_scalar(
                        cA[4][:], q3A[:], -2.0, 1.0, ALU.mult, ALU.add
                    )

                    # ---- trig ladder, U side ([128, 256] bf16 tiles)
                    def unew(tag):
                        return sb.tile([128, 2 * D], bf16, tag=tag, name=tag)

                    sU = {}; cU = {}
                    shU = unew("shU")
                    nc.scalar.activation(shU[:], pu[:], AF.Sin, scale=0.5 * W0)
                    sU[1] = unew("s1U")
                    nc.scalar.activation(sU[1][:], pu[:], AF.Sin, scale=W0)
                    qU = unew("qU")
                    nc.vector.tensor_mul(qU[:], shU[:], shU[:])
                    cU[1] = unew("c1U")
                    nc.vector.tensor_scalar(cU[1][:], qU[:], -2.0, 1.0, ALU.mult, ALU.add)
                    sU[2] = unew("s2U")
                    nc.vector.scalar_tensor_tensor(
                        sU[2][:], sU[1][:], 2.0, cU[1][:], ALU.mult, ALU.mult
                    )
                    q1U = unew("q1U")
                    nc.vector.tensor_mul(q1U[:], sU[1][:], sU[1][:])
                    cU[2] = unew("c2U")
                    nc.vector.tensor_scalar(cU[2][:], q1U[:], -2.0, 1.0, ALU.mult, ALU.add)
                    wU = unew("wU")
                    nc.vector.tensor_scalar(wU[:], q1U[:], -4.0, 3.0, ALU.mult, ALU.add)
                    sU[3] = unew("s3U")
                    nc.vector.tensor_mul(sU[3][:], wU[:], sU[1][:])
                    q2U = unew("q2U")
                    nc.vector.tensor_mul(q2U[:], cU[1][:], cU[1][:])
                    w2U = unew("w2U")
                    nc.vector.tensor_scalar(w2U[:], q2U[:], 4.0, -3.0, ALU.mult, ALU.add)
                    cU[3] = unew("c3U")
                    nc.vector.tensor_mul(cU[3][:], w2U[:], cU[1][:])
                    sU[4] = unew("s4U")
                    nc.vector.scalar_tensor_tensor(
                        sU[4][:], sU[2][:], 2.0, cU[2][:], ALU.mult, ALU.mult
                    )
                    q3U = unew("q3U")
                    nc.vector.tensor_mul(q3U[:], sU[2][:], sU[2][:])
                    cU[4] = unew("c4U")
                    nc.vector.tensor_scalar(cU[4][:], q3U[:], -2.0, 1.0, ALU.mult, ALU.add)

                    # ---- scaled stationary leaves Hs/Hc [128, 128] per (k, chunk)
                    Hs = {}; Hc = {}
                    for k in range(1, KH + 1):
                        for ch in range(2):
                            col = vsk_s[:, (k - 1) * 2 + ch : (k - 1) * 2 + ch + 1]
                            hs = sb.tile([128, D], bf16, tag=f"Hs{k}{ch}", name=f"Hs{k}{ch}")
                            nc.vector.tensor_scalar(
                                hs[:], sU[k][:, ch * D : (ch + 1) * D], col, None, ALU.mult
                            )
                            hc = sb.tile([128, D], bf16, tag=f"Hc{k}{ch}", name=f"Hc{k}{ch}")
                            nc.vector.tensor_scalar(
                                hc[:], cU[k][:, ch * D : (ch + 1) * D], col, None, ALU.mult
                            )
                            Hs[(k, ch)] = hs
                            Hc[(k, ch)] = hc

                    # ---- score matmuls: scp[d, t] = sum_k sum_s (...)
                    scp = psc.tile([D, T], f32, tag="scp", name="scp")
                    n_mm = 4 * KH
                    i_mm = 0
                    for k in range(1, KH + 1):
                        for ch in range(2):
                            nc.tensor.matmul(
                                scp[:],
                                Hc[(k, ch)][:],
                                sA[k][:, ch * T : (ch + 1) * T],
                                start=(i_mm == 0), stop=(i_mm == n_mm - 1),
                            )
                            i_mm += 1
                            nc.tensor.matmul(
                                scp[:],
                                Hs[(k, ch)][:],
                                cA[k][:, ch * T : (ch + 1) * T],
                                start=False, stop=(i_mm == n_mm - 1),
                            )
                            i_mm += 1

                    # ---- softmax over d (partitions) via tanh-exp
                    tht = sb.tile([D, T], bf16, tag="tht", name="tht")
                    nc.scalar.activation(tht[:], scp[:], AF.Tanh, scale=0.5)
                    den = sb.tile([D, T], f32, tag="den", name="den")
                    nc.vector.tensor_scalar(den[:], tht[:], -1.0, 1.0, ALU.mult, ALU.add)
                    rden = sb.tile([D, T], f32, tag="rden", name="rden")
                    nc.vector.reciprocal(rden[:], den[:])
                    Et = sb.tile([D, T], bf16, tag="Et", name="Et")
                    nc.vector.scalar_tensor_tensor(
                        Et[:], tht[:], 1.0, rden[:], ALU.add, ALU.mult
                    )
                    sums = psm.tile([1, T], f32, tag="sums", name="sums")
                    nc.tensor.matmul(sums[:], onesc_s[:], Et[:], start=True, stop=True)
                    rrf = sb.tile([1, T], f32, tag="rrf", name="rrf")
                    nc.vector.reciprocal(rrf[:], sums[:])
                    rrb = sb.tile([1, T], bf16, tag="rrb", name="rrb")
                    nc.vector.tensor_scalar(rrb[:], rrf[:], 0.0, None, ALU.add)
                    rbc = psm.tile([D, T], f32, tag="rbc", name="rbc")
                    nc.tensor.matmul(
                        rbc[:], onesr_s[0:1, 0:D], rrb[:], start=True, stop=True
                    )
                    tmp = sb.tile([D, T], bf16, tag="tmpm", name="tmpm")
                    nc.vector.tensor_mul(tmp[:], Et[:], xT[:])
                    xtT = sb.tile([D, T], bf16, tag="xtT", name="xtT")
                    nc.vector.tensor_mul(xtT[:], tmp[:], rbc[:])
                    for q in range(NC):
                        nc.sync.dma_start(
                            out=cc_in[q, b, :, :],
                            in_=xtT[:, q * TPC : (q + 1) * TPC],
                        )

            # ---------------- AllToAll ----------------
            nc.gpsimd.collective_compute(
                "AllToAll",
                mybir.AluOpType.bypass,
                replica_groups=[list(range(NC))],
                ins=[cc_in],
                outs=[cc_out],
            )

            # ---------------- stage 4 ----------------
            with (
                tc.tile_pool(name="sb4", bufs=3) as sb4,
                tc.tile_pool(name="sb4c", bufs=1) as sb4c,
                tc.tile_pool(name="ps4", bufs=2, space="PSUM") as ps4,
            ):
                xTt = sb4c.tile([D, nbB * TPC], bf16, tag="xTt", name="xTt")
                for i in range(NC):
                    nc.sync.dma_start(
                        out=xTt[:, i * nb * TPC : (i + 1) * nb * TPC].rearrange(
                            "d (b l) -> d b l", l=TPC
                        ),
                        in_=cc_out[i].rearrange("b d l -> d b l"),
                    )

                ctiles = [
                    sb4c.tile([H, TPC], f32, tag=f"c4_{i}", name=f"c4_{i}")
                    for i in range(2)
                ]
                htiles = [
                    sb4c.tile([H, TPC], bf16, tag=f"h4_{i}", name=f"h4_{i}")
                    for i in range(2)
                ]
                nc.vector.memset(ctiles[0][:], 0.0)
                nc.vector.memset(htiles[0][:], 0.0)

                for bg in range(nbB // S4G):
                    p1 = ps4.tile([128, 4 * S4G * TPC], f32, tag="p1", name="p1")
                    for c in range(4):
                        o = c * S4G * TPC
                        nc.tensor.matmul(
                            p1[:, o : o + S4G * TPC],
                            wih2t_s[:, c * 128 : (c + 1) * 128],
                            xTt[:, bg * S4G * TPC : (bg + 1) * S4G * TPC],
                            start=True, stop=False,
                            skip_group_check=True,
                        )
                        nc.tensor.matmul(
                            p1[:, o : o + S4G * TPC],
                            b2r_s[0:1, c * 128 : (c + 1) * 128],
                            onesr_s[0:1, 0 : S4G * TPC],
                            start=False, stop=False,
                            skip_group_check=True,
                        )
                    for bl in range(S4G):
                        b = bg * S4G + bl
                        hprev = htiles[b % 2]
                        cprev = ctiles[b % 2]
                        hcur = htiles[1 - b % 2]
                        ccur = ctiles[1 - b % 2]
                        for chn in range(NCH):
                            lo = chn * LCH
                            for c in range(4):
                                o = c * S4G * TPC + bl * TPC + lo
                                nc.tensor.matmul(
                                    p1[:, o : o + LCH],
                                    whh2t_s[:, c * 128 : (c + 1) * 128],
                                    hprev[:, lo : lo + LCH],
                                    start=False, stop=(c == 3),
                                    skip_group_check=True,
                                )
                            g4 = p1[:].rearrange(
                                "p (c b l) -> p c b l", c=4, b=S4G
                            )[:, :, bl, lo : lo + LCH]
                            t4 = sb4.tile([H, 4 * LCH], bf16, tag=f"t4_{chn}", name=f"t4_{chn}")
                            nc.scalar.activation(
                                t4[:].rearrange("p (c l) -> p c l", c=4),
                                g4, AF.Tanh, scale=0.5,
                            )
                            u4 = sb4.tile([H, LCH], f32, tag=f"u4_{chn}", name=f"u4_{chn}")
                            nc.vector.scalar_tensor_tensor(
                                u4[:], t4[:, 0:LCH], 1.0, t4[:, 3 * LCH : 4 * LCH],
                                ALU.add, ALU.mult,
                            )
                            v4 = sb4.tile([H, LCH], f32, tag=f"v4_{chn}", name=f"v4_{chn}")
                            nc.vector.scalar_tensor_tensor(
                                v4[:], t4[:, LCH : 2 * LCH], 1.0,
                                cprev[:, lo : lo + LCH],
                                ALU.add, ALU.mult,
                            )
                            nc.vector.scalar_tensor_tensor(
                                ccur[:, lo : lo + LCH], v4[:], 0.5, u4[:],
                                ALU.mult, ALU.add,
                            )
                            tc4 = sb4.tile([H, LCH], f32, tag=f"tc4_{chn}", name=f"tc4_{chn}")
                            nc.scalar.activation(
                                tc4[:], ccur[:, lo : lo + LCH], AF.Tanh, scale=0.5
                            )
                            nc.vector.scalar_tensor_tensor(
                                hcur[:, lo : lo + LCH], t4[:, 2 * LCH : 3 * LCH],
                                1.0, tc4[:],
                                ALU.add, ALU.mult,
                            )
                        nc.sync.dma_start(out=y_d[b], in_=hcur[:])

    nc.compile()
    return nc


def _get_nc(nb=BPC):
    key = ("v2", nb, KH, LFIT, S4G, NCH)
    if key not in _CACHE:
        _CACHE[key] = _build(nb=nb)
    return _CACHE[key]


def _prep_weights(W_ih1, b_ih1, W_hh1, b_hh1, W_we, b_we, W_ue, b_ue, W_ve, b_ve,
                  W_ih2, b_ih2, W_hh2, b_hh2):
    import ml_dtypes

    bfd = ml_dtypes.bfloat16
    f = np.float32
    om, coef = _fit_coeffs()
    b1 = (b_ih1 + b_hh1).astype(f)
    # stage 1 gate packing (i, 2g, o) for the single tanh(0.5*x) pass
    w1t = np.concatenate(
        [W_ih1[0:H].T, 2.0 * W_ih1[2 * H : 3 * H].T, W_ih1[3 * H : 4 * H].T], axis=1
    ).astype(bfd)
    b1r = np.concatenate(
        [b1[0:H], 2.0 * b1[2 * H : 3 * H], b1[3 * H : 4 * H]]
    ).reshape(1, 3 * H).astype(bfd)
    # stage 2: A uses h2=2h, c2=2c -> pre-scale 0.5; [j-half][j128, s256]
    wwet = (0.5 * W_we.T).reshape(2, H, 2 * H).astype(bfd)
    bwer = b_we.reshape(1, 2 * H).astype(bfd)
    wuet = W_ue.T.reshape(2, H, 2 * H).astype(bfd)
    buer = b_ue.reshape(1, 2 * H).astype(bfd)
    # sine-coefficient-scaled v columns: [128, (k, chunk)]
    v = W_ve[0].astype(f)
    vsk = np.empty((H, 2 * KH), f)
    for k in range(KH):
        for ch in range(2):
            vsk[:, k * 2 + ch] = coef[k] * v[ch * 128 : (ch + 1) * 128]
    affc = np.tile(np.array([[1.0, 3.0, -3.0]], f), (H, 1)).astype(f)
    onesr = np.ones((1, 512), bfd)
    onesc = np.ones((H, 1), bfd)
    # stage 4 (order i, f, o, g; g-gate doubled; hh pre-halved for h2=2h)
    perm = np.concatenate(
        [np.arange(0, H), np.arange(H, 2 * H), np.arange(3 * H, 4 * H),
         np.arange(2 * H, 3 * H)]
    )
    Wi2 = W_ih2[perm].copy()
    Wh2 = W_hh2[perm].copy()
    b2 = (b_ih2 + b_hh2)[perm].copy()
    Wi2[3 * H :] *= 2.0
    b2[3 * H :] *= 2.0
    Wh2 = 0.5 * Wh2
    Wh2[3 * H :] *= 2.0
    wih2t = Wi2.T.copy().astype(bfd)
    whh2t = Wh2.T.copy().astype(bfd)
    b2r = b2.reshape(1, 4 * H).astype(bfd)
    return dict(
        w1t=w1t, b1r=b1r, wwet=wwet, bwer=bwer, wuet=wuet, buer=buer,
        vsk=vsk, affc=affc, onesr=onesr, onesc=onesc, wih2t=wih2t,
        whh2t=whh2t, b2r=b2r,
    )


def kernel(X, W_ih1, b_ih1, W_hh1, b_hh1, W_we, b_we, W_ue, b_ue, W_ve, b_ve,
           W_ih2, b_ih2, W_hh2, b_hh2, _trace=False):
    import ml_dtypes
    from concourse.bass_utils import run_bass_kernel_spmd

    bfd = ml_dtypes.bfloat16
    X = np.asarray(X, dtype=np.float32)
    wd = _prep_weights(
        np.asarray(W_ih1), np.asarray(b_ih1), np.asarray(W_hh1), np.asarray(b_hh1),
        np.asarray(W_we), np.asarray(b_we), np.asarray(W_ue), np.asarray(b_ue),
        np.asarray(W_ve), np.asarray(b_ve), np.asarray(W_ih2), np.asarray(b_ih2),
        np.asarray(W_hh2), np.asarray(b_hh2),
    )
    nc = _get_nc()
    Xb = X.astype(bfd)
    in_maps = []
    for k in range(NC):
        xs = Xb[k * BPC : (k + 1) * BPC]
        in_maps.append({
            "xnat": np.ascontiguousarray(xs),
            "xtr": np.ascontiguousarray(xs.transpose(0, 2, 1)),
            **wd,
        })
    res = run_bass_kernel_spmd(nc, in_maps, core_ids=list(range(NC)), trace=_trace)
    out = np.empty((B, T, H), dtype=np.float32)
    for k in range(NC):
        yk = res.results[k]["y"].astype(np.float32)  # [B, H, TPC] (= 2h)
        out[:, k * TPC : (k + 1) * TPC, :] = 0.5 * yk.transpose(0, 2, 1)
    if _trace:
        kernel.last_result = res
    return out
